# revision 2
# baseline (speedup 1.0000x reference)
"""GAT message-passing kernel for Trainium2 (8 NeuronCores, Bass/Tile).

Strategy (edge-parallel graph partitioning, per the sharding hint):
the model output y = elu(sum(xo[0] * xo[1:item_len], 1)) depends only on
output rows 0..item_len-1, so only edges with dst < item_len contribute.
Python partitions those edges by dst block of 128 (core k owns dst rows
[128k, 128k+128)); every core additionally processes the dst==0 edges so
xo[0] is available locally (no collectives needed).

On device, per core:
  - gather x[src] rows via indirect DMA (node features stay replicated
    in HBM; the gather is the memory-bound core of the kernel)
  - per-edge attention logits via fused multiply+row-reduce against
    broadcast W@att_src / W@att_dst vectors
  - segment softmax numerator + scatter-add via one-hot selection
    matmuls accumulated in PSUM:  acc[j,:] += S^T @ [p * x_src | p]
  - out = (acc_u @ W) / z + bias;  xo = elu(out)
  - y_k[j] = elu(dot(xo[0], xo[j]))  (row 0 from the dst==0 block)
Python concatenates the 8 y_k slices into the final [item_len-1] vector.
"""
import math

import numpy as np

P = 128
N_CORES = 8
NEG_SLOPE = 0.2

_CACHE = {}


def _build_program(n_nodes, in_dim, out_dim, T_main, T0):
    import concourse.bass as bass
    import concourse.bacc as bacc
    import concourse.tile as tile
    import concourse.mybir as mybir
    from concourse.masks import make_identity
    from contextlib import ExitStack

    f32 = mybir.dt.float32
    i32 = mybir.dt.int32
    Alu = mybir.AluOpType
    Act = mybir.ActivationFunctionType
    IND = in_dim
    OUTD = out_dim

    nc = bacc.Bacc(
        "TRN2", target_bir_lowering=False, debug=False, num_devices=N_CORES
    )
    x_in = nc.dram_tensor("x_in", [n_nodes, IND], f32, kind="ExternalInput").ap()
    w_in = nc.dram_tensor("w_in", [IND, OUTD], f32, kind="ExternalInput").ap()
    attsrc_in = nc.dram_tensor("attsrc_in", [OUTD, 1], f32, kind="ExternalInput").ap()
    attdst_in = nc.dram_tensor("attdst_in", [OUTD, 1], f32, kind="ExternalInput").ap()
    bias_in = nc.dram_tensor("bias_in", [1, OUTD], f32, kind="ExternalInput").ap()
    esrc_in = nc.dram_tensor("esrc_in", [P, T_main], i32, kind="ExternalInput").ap()
    edst_in = nc.dram_tensor("edst_in", [P, T_main], i32, kind="ExternalInput").ap()
    bsrc_in = nc.dram_tensor("bsrc_in", [P, T0], i32, kind="ExternalInput").ap()
    bdst_in = nc.dram_tensor("bdst_in", [P, T0], i32, kind="ExternalInput").ap()
    mrows_in = nc.dram_tensor("mrows_in", [P, 1], i32, kind="ExternalInput").ap()
    brows_in = nc.dram_tensor("brows_in", [P, 1], i32, kind="ExternalInput").ap()
    y_out = nc.dram_tensor("y_out", [P, 1], f32, kind="ExternalOutput").ap()

    with tile.TileContext(nc) as tc, ExitStack() as ctx:
        const = ctx.enter_context(tc.tile_pool(name="const", bufs=1))
        idxp = ctx.enter_context(tc.tile_pool(name="idx", bufs=2))
        xgp = ctx.enter_context(tc.tile_pool(name="xg", bufs=6))
        sp = ctx.enter_context(tc.tile_pool(name="sp", bufs=3))
        scrp = ctx.enter_context(tc.tile_pool(name="scr", bufs=2))
        rhsp = ctx.enter_context(tc.tile_pool(name="rhs", bufs=3))
        smallp = ctx.enter_context(tc.tile_pool(name="small", bufs=3))
        accsbp = ctx.enter_context(tc.tile_pool(name="accsb", bufs=2))
        xop = ctx.enter_context(tc.tile_pool(name="xop", bufs=2))
        # PSUM: acc 2 + tp 2 + adp 2 + outp 2 = 8 banks
        accp = ctx.enter_context(tc.tile_pool(name="acc", bufs=2, space="PSUM"))
        tpp = ctx.enter_context(tc.tile_pool(name="tp", bufs=2, space="PSUM"))
        adp = ctx.enter_context(tc.tile_pool(name="adp", bufs=2, space="PSUM"))
        outpp = ctx.enter_context(tc.tile_pool(name="outp", bufs=2, space="PSUM"))

        # ---- constants ----
        ident = const.tile([P, P], f32, tag="ident")
        make_identity(nc, ident[:])
        iota_i = const.tile([P, P], i32, tag="iota_i")
        nc.gpsimd.iota(iota_i[:], pattern=[[1, P]], base=0, channel_multiplier=0)
        iota_f = const.tile([P, P], f32, tag="iota_f")
        nc.vector.tensor_copy(iota_f[:], iota_i[:])
        ones_row = const.tile([1, P], f32, tag="ones_row")
        nc.vector.memset(ones_row[:], 1.0)

        W0 = const.tile([P, OUTD], f32, tag="W0")
        nc.sync.dma_start(W0[:], w_in[0:P, :])
        W1 = const.tile([P, OUTD], f32, tag="W1")
        nc.sync.dma_start(W1[:], w_in[P : 2 * P, :])
        as_col = const.tile([OUTD, 1], f32, tag="as_col")
        nc.sync.dma_start(as_col[:], attsrc_in[:])
        ad_col = const.tile([OUTD, 1], f32, tag="ad_col")
        nc.sync.dma_start(ad_col[:], attdst_in[:])
        bias_row = const.tile([1, OUTD], f32, tag="bias_row")
        nc.sync.dma_start(bias_row[:], bias_in[:])

        # W^T chunks for computing w_s = W @ att_src, w_d = W @ att_dst
        WT = []
        for ci, Wc in enumerate((W0, W1)):
            wtp = tpp.tile([P, P], f32, tag="tp")
            nc.tensor.transpose(wtp[:], Wc[:], ident[:])
            wts = const.tile([P, P], f32, tag=f"WT{ci}")
            nc.vector.tensor_copy(wts[:], wtp[:])
            WT.append(wts)

        def proj_row(att_col, name):
            # returns [1, IND] sbuf row of W @ att  (length IND)
            row = const.tile([1, IND], f32, tag=f"{name}_row")
            for ci in range(2):
                colp = adp.tile([P, 1], f32, tag="adp")
                nc.tensor.matmul(
                    colp[:], lhsT=WT[ci][:], rhs=att_col[:], start=True, stop=True
                )
                cols = const.tile([P, 1], f32, tag=f"{name}_col{ci}")
                nc.vector.tensor_copy(cols[:], colp[:])
                rowp = tpp.tile([P, P], f32, tag="tp")
                nc.tensor.transpose(rowp[:1, :], cols[:], ident[:])
                nc.vector.tensor_copy(row[:, ci * P : (ci + 1) * P], rowp[:1, :P])
            return row

        ws_row = proj_row(as_col, "ws")
        wd_row = proj_row(ad_col, "wd")

        def bcast_rows(row_sb, width, name):
            # [1, width] -> [P, width] via rank-1 matmul
            bp = tpp.tile([P, IND], f32, tag="tp")
            nc.tensor.matmul(
                bp[:, :width], lhsT=ones_row[:], rhs=row_sb[:, :width],
                start=True, stop=True,
            )
            bs = const.tile([P, width], f32, tag=f"{name}_b")
            nc.vector.tensor_copy(bs[:], bp[:, :width])
            return bs

        wsb = bcast_rows(ws_row, IND, "wsb")
        wdb = bcast_rows(wd_row, IND, "wdb")
        bias_b = bcast_rows(bias_row, OUTD, "bias")

        def emit_block(rows_dram, esrc_dram, edst_dram, T, tag):
            rows_t = idxp.tile([P, 1], i32, tag="rows")
            nc.sync.dma_start(rows_t[:], rows_dram[:])
            xb = xgp.tile([P, IND], f32, tag="xb")
            nc.gpsimd.indirect_dma_start(
                out=xb[:],
                out_offset=None,
                in_=x_in[:],
                in_offset=bass.IndirectOffsetOnAxis(ap=rows_t[:, :1], axis=0),
            )
            adb = smallp.tile([P, 1], f32, tag="adb")
            scr_b = scrp.tile([P, IND], f32, tag="scr")
            nc.vector.scalar_tensor_tensor(
                out=scr_b[:], in0=xb[:], scalar=0.0, in1=wdb[:],
                op0=Alu.bypass, op1=Alu.mult, accum_out=adb[:],
            )
            esrc_t = idxp.tile([P, T], i32, tag=f"esrc{tag}")
            nc.sync.dma_start(esrc_t[:], esrc_dram[:])
            edst_t = idxp.tile([P, T], i32, tag=f"edst{tag}")
            nc.sync.dma_start(edst_t[:], edst_dram[:])

            acc = accp.tile([P, IND + 1], f32, tag="acc")
            for t in range(T):
                xg = xgp.tile([P, IND], f32, tag="xg")
                nc.gpsimd.indirect_dma_start(
                    out=xg[:],
                    out_offset=None,
                    in_=x_in[:],
                    in_offset=bass.IndirectOffsetOnAxis(
                        ap=esrc_t[:, t : t + 1], axis=0
                    ),
                )
                dstf = smallp.tile([P, 1], f32, tag="dstf")
                nc.vector.tensor_copy(dstf[:], edst_t[:, t : t + 1])
                S = sp.tile([P, P], f32, tag="S")
                nc.vector.tensor_tensor(
                    out=S[:], in0=dstf[:].to_broadcast([P, P]), in1=iota_f[:],
                    op=Alu.is_equal,
                )
                stp = tpp.tile([P, P], f32, tag="tp")
                nc.tensor.transpose(stp[:], S[:], ident[:])
                ST = sp.tile([P, P], f32, tag="ST")
                nc.vector.tensor_copy(ST[:], stp[:])
                adep = adp.tile([P, 1], f32, tag="adp")
                nc.tensor.matmul(
                    adep[:], lhsT=ST[:], rhs=adb[:], start=True, stop=True
                )
                a_s = smallp.tile([P, 1], f32, tag="a_s")
                scr = scrp.tile([P, IND], f32, tag="scr")
                nc.vector.scalar_tensor_tensor(
                    out=scr[:], in0=xg[:], scalar=0.0, in1=wsb[:],
                    op0=Alu.bypass, op1=Alu.mult, accum_out=a_s[:],
                )
                v = smallp.tile([P, 1], f32, tag="v")
                nc.vector.tensor_tensor(out=v[:], in0=a_s[:], in1=adep[:], op=Alu.add)
                lr = smallp.tile([P, 1], f32, tag="lr")
                nc.vector.tensor_scalar_mul(lr[:], v[:], NEG_SLOPE)
                e = smallp.tile([P, 1], f32, tag="e")
                nc.vector.tensor_tensor(out=e[:], in0=v[:], in1=lr[:], op=Alu.max)
                p = smallp.tile([P, 1], f32, tag="p")
                nc.scalar.activation(p[:], e[:], Act.Exp)
                rhs = rhsp.tile([P, IND + 1], f32, tag="rhs")
                nc.scalar.activation(rhs[:, 0:IND], xg[:], Act.Copy, scale=p[:])
                nc.vector.tensor_copy(rhs[:, IND : IND + 1], p[:])
                nc.tensor.matmul(
                    acc[:], lhsT=S[:], rhs=rhs[:],
                    start=(t == 0), stop=(t == T - 1), skip_group_check=True,
                )

            acc_sb = accsbp.tile([P, IND + 1], f32, tag="acc_sb")
            nc.vector.tensor_copy(acc_sb[:], acc[:])
            outp = outpp.tile([P, OUTD], f32, tag="outp")
            for ci in range(2):
                utp = tpp.tile([P, P], f32, tag="tp")
                nc.tensor.transpose(
                    utp[:], acc_sb[:, ci * P : (ci + 1) * P], ident[:]
                )
                uT = sp.tile([P, P], f32, tag="uT")
                nc.vector.tensor_copy(uT[:], utp[:])
                nc.tensor.matmul(
                    outp[:], lhsT=uT[:], rhs=(W0 if ci == 0 else W1)[:],
                    start=(ci == 0), stop=(ci == 1), skip_group_check=True,
                )
            zeps = smallp.tile([P, 1], f32, tag="zeps")
            nc.vector.tensor_scalar_add(zeps[:], acc_sb[:, IND : IND + 1], 1e-30)
            rz = smallp.tile([P, 1], f32, tag="rz")
            nc.vector.reciprocal(rz[:], zeps[:])
            outn = xop.tile([P, OUTD], f32, tag="outn")
            nc.vector.scalar_tensor_tensor(
                out=outn[:], in0=outp[:], scalar=rz[:], in1=bias_b[:],
                op0=Alu.mult, op1=Alu.add,
            )
            tneg = xop.tile([P, OUTD], f32, tag="tneg")
            nc.vector.tensor_scalar_min(tneg[:], outn[:], 0.0)
            texp = xop.tile([P, OUTD], f32, tag="texp")
            nc.scalar.activation(texp[:], tneg[:], Act.Exp)
            tpos = xop.tile([P, OUTD], f32, tag="tpos")
            nc.vector.tensor_scalar_max(tpos[:], outn[:], 0.0)
            xo = xop.tile([P, OUTD], f32, tag="xo")
            nc.vector.scalar_tensor_tensor(
                out=xo[:], in0=texp[:], scalar=-1.0, in1=tpos[:],
                op0=Alu.add, op1=Alu.add,
            )
            return xo

        xo_m = emit_block(mrows_in, esrc_in, edst_in, T_main, "m")
        xo_b = emit_block(brows_in, bsrc_in, bdst_in, T0, "b")

        xo0b = tpp.tile([P, P], f32, tag="tp")
        nc.tensor.matmul(
            xo0b[:, :OUTD], lhsT=ones_row[:], rhs=xo_b[0:1, :], start=True, stop=True
        )
        xo0s = sp.tile([P, OUTD], f32, tag="xo0s")
        nc.vector.tensor_copy(xo0s[:], xo0b[:, :OUTD])
        dscr = sp.tile([P, OUTD], f32, tag="dscr")
        d_sb = smallp.tile([P, 1], f32, tag="d")
        nc.vector.scalar_tensor_tensor(
            out=dscr[:], in0=xo_m[:], scalar=0.0, in1=xo0s[:],
            op0=Alu.bypass, op1=Alu.mult, accum_out=d_sb[:],
        )
        yneg = smallp.tile([P, 1], f32, tag="yneg")
        nc.vector.tensor_scalar_min(yneg[:], d_sb[:], 0.0)
        yexp = smallp.tile([P, 1], f32, tag="yexp")
        nc.scalar.activation(yexp[:], yneg[:], Act.Exp)
        ypos = smallp.tile([P, 1], f32, tag="ypos")
        nc.vector.tensor_scalar_max(ypos[:], d_sb[:], 0.0)
        y_sb = smallp.tile([P, 1], f32, tag="y_sb")
        nc.vector.scalar_tensor_tensor(
            out=y_sb[:], in0=yexp[:], scalar=-1.0, in1=ypos[:],
            op0=Alu.add, op1=Alu.add,
        )
        nc.sync.dma_start(y_out[:], y_sb[:])

    nc.compile()
    return nc


def _get_program(n_nodes, in_dim, out_dim, T_main, T0):
    key = (n_nodes, in_dim, out_dim, T_main, T0)
    if key not in _CACHE:
        _CACHE[key] = _build_program(n_nodes, in_dim, out_dim, T_main, T0)
    return _CACHE[key]


def _pack_edges(src, dst_local, T):
    npad = T * P - len(src)
    s = np.concatenate([src, np.zeros(npad, np.int32)])
    d = np.concatenate([dst_local, np.full(npad, P, np.int32)])
    return (
        np.ascontiguousarray(s.reshape(T, P).T),
        np.ascontiguousarray(d.reshape(T, P).T),
    )


def prepare(x, edge_index, W, att_src, att_dst, bias, item_len):
    """Python-side graph partitioning; returns (nc, in_maps, item_len)."""
    item_len = int(np.asarray(item_len))
    x = np.ascontiguousarray(np.asarray(x, np.float32))
    W = np.ascontiguousarray(np.asarray(W, np.float32))
    att_src = np.asarray(att_src, np.float32)
    att_dst = np.asarray(att_dst, np.float32)
    bias = np.asarray(bias, np.float32)
    n_nodes, in_dim = x.shape
    out_dim = W.shape[1]
    assert item_len <= N_CORES * P, "kernel supports item_len <= 1024"

    src = np.asarray(edge_index[0])
    dst = np.asarray(edge_index[1])
    keep = dst < item_len
    src_f = src[keep].astype(np.int32)
    dst_f = dst[keep].astype(np.int32)
    loops = np.arange(item_len, dtype=np.int32)
    src_f = np.concatenate([src_f, loops])
    dst_f = np.concatenate([dst_f, loops])

    blk = dst_f // P
    order = np.argsort(blk, kind="stable")
    src_f = src_f[order]
    dst_f = dst_f[order]
    blk = blk[order]
    bounds = np.searchsorted(blk, np.arange(N_CORES + 1))
    counts = np.diff(bounds)
    T_main = max(1, int(math.ceil(counts.max() / P)))
    T_main += T_main % 2  # round to even for compile-cache stability

    sel0 = dst_f == 0
    b0_src = src_f[sel0]
    b0_dst = dst_f[sel0]
    T0 = max(1, int(math.ceil(len(b0_src) / P)))
    bsrc, bdst = _pack_edges(b0_src, b0_dst, T0)
    brows = np.arange(P, dtype=np.int32).reshape(P, 1)

    nc = _get_program(n_nodes, in_dim, out_dim, T_main, T0)

    in_maps = []
    for k in range(N_CORES):
        lo, hi = bounds[k], bounds[k + 1]
        esrc, edst = _pack_edges(src_f[lo:hi], dst_f[lo:hi] - k * P, T_main)
        mrows = np.minimum(
            np.arange(k * P, (k + 1) * P, dtype=np.int32), n_nodes - 1
        ).reshape(P, 1)
        in_maps.append(
            {
                "x_in": x,
                "w_in": W,
                "attsrc_in": att_src.reshape(out_dim, 1),
                "attdst_in": att_dst.reshape(out_dim, 1),
                "bias_in": bias.reshape(1, out_dim),
                "esrc_in": esrc,
                "edst_in": edst,
                "bsrc_in": bsrc,
                "bdst_in": bdst,
                "mrows_in": mrows,
                "brows_in": brows,
            }
        )
    return nc, in_maps, item_len


def assemble(results, item_len):
    y_all = np.concatenate([results[k]["y_out"].ravel() for k in range(N_CORES)])
    return y_all[1:item_len].astype(np.float32)


def kernel(x, edge_index, W, att_src, att_dst, bias, item_len):
    from concourse import bass_utils

    nc, in_maps, item_len = prepare(
        x, edge_index, W, att_src, att_dst, bias, item_len
    )
    res = bass_utils.run_bass_kernel_spmd(nc, in_maps, core_ids=list(range(N_CORES)))
    return assemble(res.results, item_len)


# revision 3
# speedup vs baseline: 1.1856x; 1.1856x over previous
"""GAT message-passing kernel for Trainium2 (8 NeuronCores, Bass/Tile).

Strategy (edge-parallel graph partitioning, per the sharding hint):
the model output y = elu(sum(xo[0] * xo[1:item_len], 1)) depends only on
output rows 0..item_len-1, so only edges with dst < item_len contribute.
Python partitions those edges by dst block of 128 (core k owns dst rows
[128k, 128k+128)); every core additionally processes the dst==0 edges so
xo[0] is available locally (no collectives needed).

On device, per core:
  - gather x[src] rows via indirect DMA (node features stay replicated
    in HBM; the gather is the memory-bound core of the kernel)
  - per-edge attention logits via fused multiply+row-reduce against
    broadcast W@att_src / W@att_dst vectors
  - segment softmax numerator + scatter-add via one-hot selection
    matmuls accumulated in PSUM:  acc[j,:] += S^T @ [p * x_src | p]
  - out = (acc_u @ W) / z + bias;  xo = elu(out)
  - y_k[j] = elu(dot(xo[0], xo[j]))  (row 0 from the dst==0 block)
Python concatenates the 8 y_k slices into the final [item_len-1] vector.
"""
import math

import numpy as np

P = 128
N_CORES = 8
NEG_SLOPE = 0.2

_CACHE = {}


def _build_program(n_nodes, in_dim, out_dim, T_main, T0):
    import concourse.bass as bass
    import concourse.bacc as bacc
    import concourse.tile as tile
    import concourse.mybir as mybir
    from concourse.masks import make_identity
    from contextlib import ExitStack

    f32 = mybir.dt.float32
    bf16 = mybir.dt.bfloat16
    i32 = mybir.dt.int32
    Alu = mybir.AluOpType
    Act = mybir.ActivationFunctionType
    IND = in_dim
    OUTD = out_dim

    nc = bacc.Bacc(
        "TRN2", target_bir_lowering=False, debug=False, num_devices=N_CORES
    )
    x_in = nc.dram_tensor("x_in", [n_nodes, IND], f32, kind="ExternalInput").ap()
    w_in = nc.dram_tensor("w_in", [IND, OUTD], f32, kind="ExternalInput").ap()
    attsrc_in = nc.dram_tensor("attsrc_in", [OUTD, 1], f32, kind="ExternalInput").ap()
    attdst_in = nc.dram_tensor("attdst_in", [OUTD, 1], f32, kind="ExternalInput").ap()
    bias_in = nc.dram_tensor("bias_in", [1, OUTD], f32, kind="ExternalInput").ap()
    esrc_in = nc.dram_tensor("esrc_in", [P, T_main], i32, kind="ExternalInput").ap()
    edst_in = nc.dram_tensor("edst_in", [P, T_main], i32, kind="ExternalInput").ap()
    bsrc_in = nc.dram_tensor("bsrc_in", [P, T0], i32, kind="ExternalInput").ap()
    bdst_in = nc.dram_tensor("bdst_in", [P, T0], i32, kind="ExternalInput").ap()
    mrows_in = nc.dram_tensor("mrows_in", [P, 1], i32, kind="ExternalInput").ap()
    brows_in = nc.dram_tensor("brows_in", [P, 1], i32, kind="ExternalInput").ap()
    y_out = nc.dram_tensor("y_out", [P, 1], f32, kind="ExternalOutput").ap()

    with tile.TileContext(nc) as tc, ExitStack() as ctx:
        const = ctx.enter_context(tc.tile_pool(name="const", bufs=1))
        idxp = ctx.enter_context(tc.tile_pool(name="idx", bufs=2))
        xgp = ctx.enter_context(tc.tile_pool(name="xg", bufs=6))
        sp = ctx.enter_context(tc.tile_pool(name="sp", bufs=3))
        scrp = ctx.enter_context(tc.tile_pool(name="scr", bufs=2))
        rhsp = ctx.enter_context(tc.tile_pool(name="rhs", bufs=3))
        smallp = ctx.enter_context(tc.tile_pool(name="small", bufs=3))
        accsbp = ctx.enter_context(tc.tile_pool(name="accsb", bufs=2))
        xop = ctx.enter_context(tc.tile_pool(name="xop", bufs=2))
        # PSUM: acc 2 + tp 2 + adp 2 + outp 2 = 8 banks
        accp = ctx.enter_context(tc.tile_pool(name="acc", bufs=2, space="PSUM"))
        tpp = ctx.enter_context(tc.tile_pool(name="tp", bufs=2, space="PSUM"))
        adp = ctx.enter_context(tc.tile_pool(name="adp", bufs=2, space="PSUM"))
        outpp = ctx.enter_context(tc.tile_pool(name="outp", bufs=2, space="PSUM"))

        # ---- constants ----
        ident = const.tile([P, P], f32, tag="ident")
        make_identity(nc, ident[:])
        iota_i = const.tile([P, P], i32, tag="iota_i")
        nc.gpsimd.iota(iota_i[:], pattern=[[1, P]], base=0, channel_multiplier=0)
        iota_f = const.tile([P, P], f32, tag="iota_f")
        nc.vector.tensor_copy(iota_f[:], iota_i[:])
        ones_row = const.tile([1, P], f32, tag="ones_row")
        nc.vector.memset(ones_row[:], 1.0)

        W0 = const.tile([P, OUTD], f32, tag="W0")
        nc.sync.dma_start(W0[:], w_in[0:P, :])
        W1 = const.tile([P, OUTD], f32, tag="W1")
        nc.sync.dma_start(W1[:], w_in[P : 2 * P, :])
        as_col = const.tile([OUTD, 1], f32, tag="as_col")
        nc.sync.dma_start(as_col[:], attsrc_in[:])
        ad_col = const.tile([OUTD, 1], f32, tag="ad_col")
        nc.sync.dma_start(ad_col[:], attdst_in[:])
        bias_row = const.tile([1, OUTD], f32, tag="bias_row")
        nc.sync.dma_start(bias_row[:], bias_in[:])

        # W^T chunks for computing w_s = W @ att_src, w_d = W @ att_dst
        WT = []
        for ci, Wc in enumerate((W0, W1)):
            wtp = tpp.tile([P, P], f32, tag="tp")
            nc.tensor.transpose(wtp[:], Wc[:], ident[:])
            wts = const.tile([P, P], f32, tag=f"WT{ci}")
            nc.vector.tensor_copy(wts[:], wtp[:])
            WT.append(wts)

        def proj_row(att_col, name):
            # returns [1, IND] sbuf row of W @ att  (length IND)
            row = const.tile([1, IND], f32, tag=f"{name}_row")
            for ci in range(2):
                colp = adp.tile([P, 1], f32, tag="adp")
                nc.tensor.matmul(
                    colp[:], lhsT=WT[ci][:], rhs=att_col[:], start=True, stop=True
                )
                cols = const.tile([P, 1], f32, tag=f"{name}_col{ci}")
                nc.vector.tensor_copy(cols[:], colp[:])
                rowp = tpp.tile([P, P], f32, tag="tp")
                nc.tensor.transpose(rowp[:1, :], cols[:], ident[:])
                nc.vector.tensor_copy(row[:, ci * P : (ci + 1) * P], rowp[:1, :P])
            return row

        ws_row = proj_row(as_col, "ws")
        wd_row = proj_row(ad_col, "wd")

        def bcast_rows(row_sb, width, name):
            # [1, width] -> [P, width] via rank-1 matmul
            bp = tpp.tile([P, IND], f32, tag="tp")
            nc.tensor.matmul(
                bp[:, :width], lhsT=ones_row[:], rhs=row_sb[:, :width],
                start=True, stop=True,
            )
            bs = const.tile([P, width], f32, tag=f"{name}_b")
            nc.vector.tensor_copy(bs[:], bp[:, :width])
            return bs

        wsb = bcast_rows(ws_row, IND, "wsb")
        wdb = bcast_rows(wd_row, IND, "wdb")
        bias_b = bcast_rows(bias_row, OUTD, "bias")

        def emit_block(rows_dram, esrc_dram, edst_dram, T, tag):
            rows_t = idxp.tile([P, 1], i32, tag="rows")
            nc.sync.dma_start(rows_t[:], rows_dram[:])
            xb = xgp.tile([P, IND], f32, tag="xb")
            nc.gpsimd.indirect_dma_start(
                out=xb[:],
                out_offset=None,
                in_=x_in[:],
                in_offset=bass.IndirectOffsetOnAxis(ap=rows_t[:, :1], axis=0),
            )
            adb = smallp.tile([P, 1], f32, tag="adb")
            scr_b = scrp.tile([P, IND], f32, tag="scr")
            nc.vector.scalar_tensor_tensor(
                out=scr_b[:], in0=xb[:], scalar=0.0, in1=wdb[:],
                op0=Alu.bypass, op1=Alu.mult, accum_out=adb[:],
            )
            # broadcast a_d along partitions: adb_b[e, j] = a_d[j]
            adrp = tpp.tile([P, P], f32, tag="tp")
            nc.tensor.transpose(adrp[:1, :], adb[:], ident[:])
            adr = const.tile([1, P], f32, tag=f"adr{tag}")
            nc.vector.tensor_copy(adr[:], adrp[:1, :P])
            adbp = tpp.tile([P, P], f32, tag="tp")
            nc.tensor.matmul(
                adbp[:], lhsT=ones_row[:], rhs=adr[:], start=True, stop=True
            )
            adb_b = const.tile([P, P], f32, tag=f"adb_b{tag}")
            nc.vector.tensor_copy(adb_b[:], adbp[:])
            esrc_t = idxp.tile([P, T], i32, tag=f"esrc{tag}")
            nc.sync.dma_start(esrc_t[:], esrc_dram[:])
            edst_t = idxp.tile([P, T], i32, tag=f"edst{tag}")
            nc.sync.dma_start(edst_t[:], edst_dram[:])
            dstf_all = idxp.tile([P, T], f32, tag=f"dstf{tag}")
            nc.vector.tensor_copy(dstf_all[:], edst_t[:])

            acc = accp.tile([P, IND + 1], f32, tag="acc")
            for t in range(T):
                xg = xgp.tile([P, IND], f32, tag="xg")
                nc.gpsimd.indirect_dma_start(
                    out=xg[:],
                    out_offset=None,
                    in_=x_in[:],
                    in_offset=bass.IndirectOffsetOnAxis(
                        ap=esrc_t[:, t : t + 1], axis=0
                    ),
                )
                S = sp.tile([P, P], bf16, tag="S")
                nc.vector.tensor_tensor(
                    out=S[:], in0=dstf_all[:, t : t + 1].to_broadcast([P, P]),
                    in1=iota_f[:], op=Alu.is_equal,
                )
                ad_e = smallp.tile([P, 1], f32, tag="ad_e")
                scr2 = sp.tile([P, P], f32, tag="scr2")
                nc.vector.scalar_tensor_tensor(
                    out=scr2[:], in0=S[:], scalar=0.0, in1=adb_b[:],
                    op0=Alu.bypass, op1=Alu.mult, accum_out=ad_e[:],
                )
                a_s = smallp.tile([P, 1], f32, tag="a_s")
                scr = scrp.tile([P, IND], f32, tag="scr")
                nc.vector.scalar_tensor_tensor(
                    out=scr[:], in0=xg[:], scalar=0.0, in1=wsb[:],
                    op0=Alu.bypass, op1=Alu.mult, accum_out=a_s[:],
                )
                v = smallp.tile([P, 1], f32, tag="v")
                nc.vector.tensor_tensor(out=v[:], in0=a_s[:], in1=ad_e[:], op=Alu.add)
                lr = smallp.tile([P, 1], f32, tag="lr")
                nc.vector.tensor_scalar_mul(lr[:], v[:], NEG_SLOPE)
                e = smallp.tile([P, 1], f32, tag="e")
                nc.vector.tensor_tensor(out=e[:], in0=v[:], in1=lr[:], op=Alu.max)
                p = smallp.tile([P, 1], f32, tag="p")
                nc.scalar.activation(p[:], e[:], Act.Exp)
                rhs = rhsp.tile([P, IND + 1], bf16, tag="rhs")
                nc.scalar.activation(rhs[:, 0:IND], xg[:], Act.Copy, scale=p[:])
                nc.vector.tensor_copy(rhs[:, IND : IND + 1], p[:])
                nc.tensor.matmul(
                    acc[:], lhsT=S[:], rhs=rhs[:],
                    start=(t == 0), stop=(t == T - 1), skip_group_check=True,
                )

            acc_sb = accsbp.tile([P, IND + 1], f32, tag="acc_sb")
            nc.vector.tensor_copy(acc_sb[:], acc[:])
            outp = outpp.tile([P, OUTD], f32, tag="outp")
            for ci in range(2):
                utp = tpp.tile([P, P], f32, tag="tp")
                nc.tensor.transpose(
                    utp[:], acc_sb[:, ci * P : (ci + 1) * P], ident[:]
                )
                uT = sp.tile([P, P], f32, tag="uT")
                nc.vector.tensor_copy(uT[:], utp[:])
                nc.tensor.matmul(
                    outp[:], lhsT=uT[:], rhs=(W0 if ci == 0 else W1)[:],
                    start=(ci == 0), stop=(ci == 1), skip_group_check=True,
                )
            zeps = smallp.tile([P, 1], f32, tag="zeps")
            nc.vector.tensor_scalar_add(zeps[:], acc_sb[:, IND : IND + 1], 1e-30)
            rz = smallp.tile([P, 1], f32, tag="rz")
            nc.vector.reciprocal(rz[:], zeps[:])
            outn = xop.tile([P, OUTD], f32, tag="outn")
            nc.vector.scalar_tensor_tensor(
                out=outn[:], in0=outp[:], scalar=rz[:], in1=bias_b[:],
                op0=Alu.mult, op1=Alu.add,
            )
            tneg = xop.tile([P, OUTD], f32, tag="tneg")
            nc.vector.tensor_scalar_min(tneg[:], outn[:], 0.0)
            texp = xop.tile([P, OUTD], f32, tag="texp")
            nc.scalar.activation(texp[:], tneg[:], Act.Exp)
            tpos = xop.tile([P, OUTD], f32, tag="tpos")
            nc.vector.tensor_scalar_max(tpos[:], outn[:], 0.0)
            xo = xop.tile([P, OUTD], f32, tag="xo")
            nc.vector.scalar_tensor_tensor(
                out=xo[:], in0=texp[:], scalar=-1.0, in1=tpos[:],
                op0=Alu.add, op1=Alu.add,
            )
            return xo

        xo_m = emit_block(mrows_in, esrc_in, edst_in, T_main, "m")
        xo_b = emit_block(brows_in, bsrc_in, bdst_in, T0, "b")

        xo0b = tpp.tile([P, P], f32, tag="tp")
        nc.tensor.matmul(
            xo0b[:, :OUTD], lhsT=ones_row[:], rhs=xo_b[0:1, :], start=True, stop=True
        )
        xo0s = sp.tile([P, OUTD], f32, tag="xo0s")
        nc.vector.tensor_copy(xo0s[:], xo0b[:, :OUTD])
        dscr = sp.tile([P, OUTD], f32, tag="dscr")
        d_sb = smallp.tile([P, 1], f32, tag="d")
        nc.vector.scalar_tensor_tensor(
            out=dscr[:], in0=xo_m[:], scalar=0.0, in1=xo0s[:],
            op0=Alu.bypass, op1=Alu.mult, accum_out=d_sb[:],
        )
        yneg = smallp.tile([P, 1], f32, tag="yneg")
        nc.vector.tensor_scalar_min(yneg[:], d_sb[:], 0.0)
        yexp = smallp.tile([P, 1], f32, tag="yexp")
        nc.scalar.activation(yexp[:], yneg[:], Act.Exp)
        ypos = smallp.tile([P, 1], f32, tag="ypos")
        nc.vector.tensor_scalar_max(ypos[:], d_sb[:], 0.0)
        y_sb = smallp.tile([P, 1], f32, tag="y_sb")
        nc.vector.scalar_tensor_tensor(
            out=y_sb[:], in0=yexp[:], scalar=-1.0, in1=ypos[:],
            op0=Alu.add, op1=Alu.add,
        )
        nc.sync.dma_start(y_out[:], y_sb[:])

    nc.compile()
    return nc


def _get_program(n_nodes, in_dim, out_dim, T_main, T0):
    key = (n_nodes, in_dim, out_dim, T_main, T0)
    if key not in _CACHE:
        _CACHE[key] = _build_program(n_nodes, in_dim, out_dim, T_main, T0)
    return _CACHE[key]


def _pack_edges(src, dst_local, T):
    npad = T * P - len(src)
    s = np.concatenate([src, np.zeros(npad, np.int32)])
    d = np.concatenate([dst_local, np.full(npad, P, np.int32)])
    return (
        np.ascontiguousarray(s.reshape(T, P).T),
        np.ascontiguousarray(d.reshape(T, P).T),
    )


def prepare(x, edge_index, W, att_src, att_dst, bias, item_len):
    """Python-side graph partitioning; returns (nc, in_maps, item_len)."""
    item_len = int(np.asarray(item_len))
    x = np.ascontiguousarray(np.asarray(x, np.float32))
    W = np.ascontiguousarray(np.asarray(W, np.float32))
    att_src = np.asarray(att_src, np.float32)
    att_dst = np.asarray(att_dst, np.float32)
    bias = np.asarray(bias, np.float32)
    n_nodes, in_dim = x.shape
    out_dim = W.shape[1]
    assert item_len <= N_CORES * P, "kernel supports item_len <= 1024"

    src = np.asarray(edge_index[0])
    dst = np.asarray(edge_index[1])
    keep = dst < item_len
    src_f = src[keep].astype(np.int32)
    dst_f = dst[keep].astype(np.int32)
    loops = np.arange(item_len, dtype=np.int32)
    src_f = np.concatenate([src_f, loops])
    dst_f = np.concatenate([dst_f, loops])

    blk = dst_f // P
    order = np.argsort(blk, kind="stable")
    src_f = src_f[order]
    dst_f = dst_f[order]
    blk = blk[order]
    bounds = np.searchsorted(blk, np.arange(N_CORES + 1))
    counts = np.diff(bounds)
    T_main = max(1, int(math.ceil(counts.max() / P)))
    T_main += T_main % 2  # round to even for compile-cache stability

    sel0 = dst_f == 0
    b0_src = src_f[sel0]
    b0_dst = dst_f[sel0]
    T0 = max(1, int(math.ceil(len(b0_src) / P)))
    bsrc, bdst = _pack_edges(b0_src, b0_dst, T0)
    brows = np.arange(P, dtype=np.int32).reshape(P, 1)

    nc = _get_program(n_nodes, in_dim, out_dim, T_main, T0)

    in_maps = []
    for k in range(N_CORES):
        lo, hi = bounds[k], bounds[k + 1]
        esrc, edst = _pack_edges(src_f[lo:hi], dst_f[lo:hi] - k * P, T_main)
        mrows = np.minimum(
            np.arange(k * P, (k + 1) * P, dtype=np.int32), n_nodes - 1
        ).reshape(P, 1)
        in_maps.append(
            {
                "x_in": x,
                "w_in": W,
                "attsrc_in": att_src.reshape(out_dim, 1),
                "attdst_in": att_dst.reshape(out_dim, 1),
                "bias_in": bias.reshape(1, out_dim),
                "esrc_in": esrc,
                "edst_in": edst,
                "bsrc_in": bsrc,
                "bdst_in": bdst,
                "mrows_in": mrows,
                "brows_in": brows,
            }
        )
    return nc, in_maps, item_len


def assemble(results, item_len):
    y_all = np.concatenate([results[k]["y_out"].ravel() for k in range(N_CORES)])
    return y_all[1:item_len].astype(np.float32)


def kernel(x, edge_index, W, att_src, att_dst, bias, item_len):
    from concourse import bass_utils

    nc, in_maps, item_len = prepare(
        x, edge_index, W, att_src, att_dst, bias, item_len
    )
    res = bass_utils.run_bass_kernel_spmd(nc, in_maps, core_ids=list(range(N_CORES)))
    return assemble(res.results, item_len)


# revision 5
# speedup vs baseline: 1.4286x; 1.2050x over previous
"""GAT message-passing kernel for Trainium2 (8 NeuronCores, Bass/Tile).

Strategy (edge-parallel graph partitioning, per the sharding hint):
the model output y = elu(sum(xo[0] * xo[1:item_len], 1)) depends only on
output rows 0..item_len-1, so only edges with dst < item_len contribute.
Python partitions those edges by dst block of 128 (core k owns dst rows
[128k, 128k+128)); every core additionally processes the dst==0 edges so
xo[0] is available locally (no collectives needed).

On device, per core:
  - gather x[src] rows via indirect DMA (node features stay replicated
    in HBM; the gather is the memory-bound core of the kernel)
  - per-edge attention logits via fused multiply+row-reduce against
    broadcast W@att_src / W@att_dst vectors
  - segment softmax numerator + scatter-add via one-hot selection
    matmuls accumulated in PSUM:  acc[j,:] += S^T @ [p * x_src | p]
    (the gathered tile carries a constant ones column so one ACT copy
    with per-edge scale p produces the whole matmul rhs)
  - out = (acc_u @ W) / z + bias;  xo = elu(out)
  - y_k[j] = elu(dot(xo[0], xo[j]))  (row 0 from the dst==0 block)
Python concatenates the 8 y_k slices into the final [item_len-1] vector.
"""
import math

import numpy as np

P = 128
N_CORES = 8
NEG_SLOPE = 0.2

_CACHE = {}


def _build_program(n_nodes, in_dim, out_dim, T_main, T0):
    import concourse.bass as bass
    import concourse.bacc as bacc
    import concourse.tile as tile
    import concourse.mybir as mybir
    from concourse.masks import make_identity
    from contextlib import ExitStack

    f32 = mybir.dt.float32
    bf16 = mybir.dt.bfloat16
    i32 = mybir.dt.int32
    Alu = mybir.AluOpType
    Act = mybir.ActivationFunctionType
    IND = in_dim
    OUTD = out_dim

    nc = bacc.Bacc(
        "TRN2", target_bir_lowering=False, debug=False, num_devices=N_CORES
    )
    x_in = nc.dram_tensor("x_in", [n_nodes, IND], f32, kind="ExternalInput").ap()
    w_in = nc.dram_tensor("w_in", [IND, OUTD], f32, kind="ExternalInput").ap()
    # att_src | att_dst as columns
    av_in = nc.dram_tensor("av_in", [OUTD, 2], f32, kind="ExternalInput").ap()
    bias_in = nc.dram_tensor("bias_in", [1, OUTD], f32, kind="ExternalInput").ap()
    # esrc || edst packed [P, 2T]
    eidx_in = nc.dram_tensor(
        "eidx_in", [P, 2 * T_main], i32, kind="ExternalInput"
    ).ap()
    bidx_in = nc.dram_tensor("bidx_in", [P, 2 * T0], i32, kind="ExternalInput").ap()
    # main rows | block-B rows
    rows_in = nc.dram_tensor("rows_in", [P, 2], i32, kind="ExternalInput").ap()
    y_out = nc.dram_tensor("y_out", [P, 1], f32, kind="ExternalOutput").ap()

    with tile.TileContext(nc) as tc, ExitStack() as ctx:
        const = ctx.enter_context(tc.tile_pool(name="const", bufs=1))
        idxp = ctx.enter_context(tc.tile_pool(name="idx", bufs=1))
        xgp = ctx.enter_context(tc.tile_pool(name="xg", bufs=T_main + T0 + 2))
        sp = ctx.enter_context(tc.tile_pool(name="sp", bufs=3))
        scrp = ctx.enter_context(tc.tile_pool(name="scr", bufs=2))
        rhsp = ctx.enter_context(tc.tile_pool(name="rhs", bufs=3))
        smallp = ctx.enter_context(tc.tile_pool(name="small", bufs=3))
        accsbp = ctx.enter_context(tc.tile_pool(name="accsb", bufs=2))
        xop = ctx.enter_context(tc.tile_pool(name="xop", bufs=2))
        # PSUM banks: acc 2 + tp 2 + adp 2 + outp 2 = 8
        accp = ctx.enter_context(tc.tile_pool(name="acc", bufs=2, space="PSUM"))
        tpp = ctx.enter_context(tc.tile_pool(name="tp", bufs=2, space="PSUM"))
        adp = ctx.enter_context(tc.tile_pool(name="adp", bufs=2, space="PSUM"))
        outpp = ctx.enter_context(tc.tile_pool(name="outp", bufs=2, space="PSUM"))

        # ---- index DMAs first: the gather stream depends only on these ----
        eidx_t = idxp.tile([P, 2 * T_main], i32, tag="eidx")
        nc.sync.dma_start(eidx_t[:], eidx_in[:])
        bidx_t = idxp.tile([P, 2 * T0], i32, tag="bidx")
        nc.sync.dma_start(bidx_t[:], bidx_in[:])
        rows_t = idxp.tile([P, 2], i32, tag="rows")
        nc.sync.dma_start(rows_t[:], rows_in[:])

        # ---- constants ----
        ident = const.tile([P, P], f32, tag="ident")
        make_identity(nc, ident[:])
        iota_i = const.tile([P, P], i32, tag="iota_i")
        nc.gpsimd.iota(iota_i[:], pattern=[[1, P]], base=0, channel_multiplier=0)
        iota_f = const.tile([P, P], f32, tag="iota_f")
        nc.vector.tensor_copy(iota_f[:], iota_i[:])
        ones_row = const.tile([1, P], f32, tag="ones_row")
        nc.vector.memset(ones_row[:], 1.0)

        W0 = const.tile([P, OUTD], f32, tag="W0")
        nc.sync.dma_start(W0[:], w_in[0:P, :])
        W1 = const.tile([P, OUTD], f32, tag="W1")
        nc.sync.dma_start(W1[:], w_in[P : 2 * P, :])
        W0h = const.tile([P, OUTD], bf16, tag="W0h")
        nc.vector.tensor_copy(W0h[:], W0[:])
        W1h = const.tile([P, OUTD], bf16, tag="W1h")
        nc.vector.tensor_copy(W1h[:], W1[:])
        av_col = const.tile([OUTD, 2], f32, tag="av_col")
        nc.sync.dma_start(av_col[:], av_in[:])
        bias_row = const.tile([1, OUTD], f32, tag="bias_row")
        nc.sync.dma_start(bias_row[:], bias_in[:])

        # W^T chunks to compute w_s = W @ att_src, w_d = W @ att_dst
        WT = []
        for ci, Wc in enumerate((W0, W1)):
            wtp = tpp.tile([P, P], f32, tag="tp")
            nc.tensor.transpose(wtp[:], Wc[:], ident[:])
            wts = const.tile([P, P], f32, tag=f"WT{ci}")
            nc.vector.tensor_copy(wts[:], wtp[:])
            WT.append(wts)

        def proj_row(att_col, name):
            # [1, IND] row of W @ att
            row = const.tile([1, IND], f32, tag=f"{name}_row")
            for ci in range(2):
                colp = adp.tile([P, 1], f32, tag="adp")
                nc.tensor.matmul(
                    colp[:], lhsT=WT[ci][:], rhs=att_col, start=True, stop=True
                )
                cols = const.tile([P, 1], f32, tag=f"{name}_col{ci}")
                nc.vector.tensor_copy(cols[:], colp[:])
                rowp = tpp.tile([P, P], f32, tag="tp")
                nc.tensor.transpose(rowp[:1, :], cols[:], ident[:])
                nc.vector.tensor_copy(row[:, ci * P : (ci + 1) * P], rowp[:1, :P])
            return row

        ws_row = proj_row(av_col[:, 0:1], "ws")
        wd_row = proj_row(av_col[:, 1:2], "wd")

        def bcast_rows(row_sb, width, name):
            # [1, width] -> [P, width] via rank-1 matmul
            bp = tpp.tile([P, IND], f32, tag="tp")
            nc.tensor.matmul(
                bp[:, :width], lhsT=ones_row[:], rhs=row_sb[:, :width],
                start=True, stop=True,
            )
            bs = const.tile([P, width], f32, tag=f"{name}_b")
            nc.vector.tensor_copy(bs[:], bp[:, :width])
            return bs

        wsb = bcast_rows(ws_row, IND, "wsb")
        wdb = bcast_rows(wd_row, IND, "wdb")
        bias_b = bcast_rows(bias_row, OUTD, "bias")

        def emit_block(rows_ap, idx_t, T, tag, self_loop_last=False):
            xb = xgp.tile([P, IND + 1], f32, tag="xb")
            nc.vector.memset(xb[:, IND : IND + 1], 1.0)
            nc.gpsimd.indirect_dma_start(
                out=xb[:, 0:IND],
                out_offset=None,
                in_=x_in[:],
                in_offset=bass.IndirectOffsetOnAxis(ap=rows_ap, axis=0),
            )
            adb = smallp.tile([P, 1], f32, tag="adb")
            scr_b = scrp.tile([P, IND], bf16, tag="scr")
            nc.vector.scalar_tensor_tensor(
                out=scr_b[:], in0=xb[:, 0:IND], scalar=0.0, in1=wdb[:],
                op0=Alu.bypass, op1=Alu.mult, accum_out=adb[:],
            )
            # broadcast a_d along partitions: adb_b[e, j] = a_d[j]
            adrp = tpp.tile([P, P], f32, tag="tp")
            nc.tensor.transpose(adrp[:1, :], adb[:], ident[:])
            adr = const.tile([1, P], f32, tag=f"adr{tag}")
            nc.vector.tensor_copy(adr[:], adrp[:1, :P])
            adbp = tpp.tile([P, P], f32, tag="tp")
            nc.tensor.matmul(
                adbp[:], lhsT=ones_row[:], rhs=adr[:], start=True, stop=True
            )
            adb_b = const.tile([P, P], f32, tag=f"adb_b{tag}")
            nc.vector.tensor_copy(adb_b[:], adbp[:])
            dstf_all = idxp.tile([P, T], f32, tag=f"dstf{tag}")
            nc.vector.tensor_copy(dstf_all[:], idx_t[:, T : 2 * T])

            acc = accp.tile([P, IND + 1], f32, tag="acc")
            for t in range(T):
                if self_loop_last and t == T - 1:
                    xg = xb
                else:
                    xg = xgp.tile([P, IND + 1], f32, tag="xg")
                    nc.vector.memset(xg[:, IND : IND + 1], 1.0)
                    nc.gpsimd.indirect_dma_start(
                        out=xg[:, 0:IND],
                        out_offset=None,
                        in_=x_in[:],
                        in_offset=bass.IndirectOffsetOnAxis(
                            ap=idx_t[:, t : t + 1], axis=0
                        ),
                    )
                dcol = dstf_all[:, t : t + 1]
                S = sp.tile([P, P], bf16, tag="S")
                nc.vector.tensor_scalar(
                    out=S[:], in0=iota_f[:], scalar1=dcol, scalar2=None,
                    op0=Alu.is_equal,
                )
                ad_e = smallp.tile([P, 1], f32, tag="ad_e")
                scr2 = sp.tile([P, P], bf16, tag="scr2")
                nc.vector.scalar_tensor_tensor(
                    out=scr2[:], in0=iota_f[:], scalar=dcol, in1=adb_b[:],
                    op0=Alu.is_equal, op1=Alu.mult, accum_out=ad_e[:],
                )
                a_s = smallp.tile([P, 1], f32, tag="a_s")
                scr = scrp.tile([P, IND], bf16, tag="scr")
                nc.vector.scalar_tensor_tensor(
                    out=scr[:], in0=xg[:, 0:IND], scalar=0.0, in1=wsb[:],
                    op0=Alu.bypass, op1=Alu.mult, accum_out=a_s[:],
                )
                v = smallp.tile([P, 1], f32, tag="v")
                nc.vector.tensor_tensor(out=v[:], in0=a_s[:], in1=ad_e[:], op=Alu.add)
                e = smallp.tile([P, 1], f32, tag="e")
                nc.vector.scalar_tensor_tensor(
                    out=e[:], in0=v[:], scalar=NEG_SLOPE, in1=v[:],
                    op0=Alu.mult, op1=Alu.max,
                )
                p = smallp.tile([P, 1], f32, tag="p")
                nc.scalar.activation(p[:], e[:], Act.Exp)
                rhs = rhsp.tile([P, IND + 1], bf16, tag="rhs")
                nc.scalar.activation(rhs[:], xg[:], Act.Copy, scale=p[:])
                nc.tensor.matmul(
                    acc[:], lhsT=S[:], rhs=rhs[:],
                    start=(t == 0), stop=(t == T - 1), skip_group_check=True,
                )

            acc_sb = accsbp.tile([P, IND + 1], f32, tag="acc_sb")
            nc.vector.tensor_copy(acc_sb[:], acc[:])
            outp = outpp.tile([P, OUTD], f32, tag="outp")
            for ci in range(2):
                utp = tpp.tile([P, P], f32, tag="tp")
                nc.tensor.transpose(
                    utp[:], acc_sb[:, ci * P : (ci + 1) * P], ident[:]
                )
                uT = sp.tile([P, P], bf16, tag="uT")
                nc.vector.tensor_copy(uT[:], utp[:])
                nc.tensor.matmul(
                    outp[:], lhsT=uT[:], rhs=(W0h if ci == 0 else W1h)[:],
                    start=(ci == 0), stop=(ci == 1), skip_group_check=True,
                )
            zeps = smallp.tile([P, 1], f32, tag="zeps")
            nc.vector.tensor_scalar_add(zeps[:], acc_sb[:, IND : IND + 1], 1e-30)
            rz = smallp.tile([P, 1], f32, tag="rz")
            nc.vector.reciprocal(rz[:], zeps[:])
            outn = xop.tile([P, OUTD], f32, tag="outn")
            nc.vector.scalar_tensor_tensor(
                out=outn[:], in0=outp[:], scalar=rz[:], in1=bias_b[:],
                op0=Alu.mult, op1=Alu.add,
            )
            tneg = xop.tile([P, OUTD], f32, tag="tneg")
            nc.vector.tensor_scalar_min(tneg[:], outn[:], 0.0)
            texp = xop.tile([P, OUTD], f32, tag="texp")
            nc.scalar.activation(texp[:], tneg[:], Act.Exp)
            tpos = xop.tile([P, OUTD], f32, tag="tpos")
            nc.vector.tensor_scalar_max(tpos[:], outn[:], 0.0)
            xo = xop.tile([P, OUTD], f32, tag="xo")
            nc.vector.scalar_tensor_tensor(
                out=xo[:], in0=texp[:], scalar=-1.0, in1=tpos[:],
                op0=Alu.add, op1=Alu.add,
            )
            return xo

        xo_b = emit_block(rows_t[:, 1:2], bidx_t, T0, "b")
        xo_m = emit_block(rows_t[:, 0:1], eidx_t, T_main, "m",
                          self_loop_last=True)

        xo0b = tpp.tile([P, P], f32, tag="tp")
        nc.tensor.matmul(
            xo0b[:, :OUTD], lhsT=ones_row[:], rhs=xo_b[0:1, :], start=True, stop=True
        )
        xo0s = sp.tile([P, OUTD], f32, tag="xo0s")
        nc.vector.tensor_copy(xo0s[:], xo0b[:, :OUTD])
        dscr = sp.tile([P, OUTD], bf16, tag="dscr")
        d_sb = smallp.tile([P, 1], f32, tag="d")
        nc.vector.scalar_tensor_tensor(
            out=dscr[:], in0=xo_m[:], scalar=0.0, in1=xo0s[:],
            op0=Alu.bypass, op1=Alu.mult, accum_out=d_sb[:],
        )
        yneg = smallp.tile([P, 1], f32, tag="yneg")
        nc.vector.tensor_scalar_min(yneg[:], d_sb[:], 0.0)
        yexp = smallp.tile([P, 1], f32, tag="yexp")
        nc.scalar.activation(yexp[:], yneg[:], Act.Exp)
        ypos = smallp.tile([P, 1], f32, tag="ypos")
        nc.vector.tensor_scalar_max(ypos[:], d_sb[:], 0.0)
        y_sb = smallp.tile([P, 1], f32, tag="y_sb")
        nc.vector.scalar_tensor_tensor(
            out=y_sb[:], in0=yexp[:], scalar=-1.0, in1=ypos[:],
            op0=Alu.add, op1=Alu.add,
        )
        nc.sync.dma_start(y_out[:], y_sb[:])

    nc.compile()
    return nc


def _get_program(n_nodes, in_dim, out_dim, T_main, T0):
    key = (n_nodes, in_dim, out_dim, T_main, T0)
    if key not in _CACHE:
        _CACHE[key] = _build_program(n_nodes, in_dim, out_dim, T_main, T0)
    return _CACHE[key]


def _pack_edges(src, dst_local, T):
    npad = T * P - len(src)
    s = np.concatenate([src, np.zeros(npad, np.int32)])
    d = np.concatenate([dst_local, np.full(npad, P, np.int32)])
    return np.concatenate(
        [
            np.ascontiguousarray(s.reshape(T, P).T),
            np.ascontiguousarray(d.reshape(T, P).T),
        ],
        axis=1,
    )


def prepare(x, edge_index, W, att_src, att_dst, bias, item_len):
    """Python-side graph partitioning; returns (nc, in_maps, item_len)."""
    item_len = int(np.asarray(item_len))
    x = np.ascontiguousarray(np.asarray(x, np.float32))
    W = np.ascontiguousarray(np.asarray(W, np.float32))
    att_src = np.asarray(att_src, np.float32)
    att_dst = np.asarray(att_dst, np.float32)
    bias = np.asarray(bias, np.float32)
    n_nodes, in_dim = x.shape
    out_dim = W.shape[1]
    assert item_len <= N_CORES * P, "kernel supports item_len <= 1024"

    src = np.asarray(edge_index[0])
    dst = np.asarray(edge_index[1])
    keep = dst < item_len
    src_f = src[keep].astype(np.int32)
    dst_f = dst[keep].astype(np.int32)
    loops = np.arange(item_len, dtype=np.int32)
    src_all = np.concatenate([src_f, loops])
    dst_all = np.concatenate([dst_f, loops])

    blk = dst_f // P  # graph edges only; self-loop tile appended per core
    order = np.argsort(blk, kind="stable")
    src_f = src_f[order]
    dst_f = dst_f[order]
    blk = blk[order]
    bounds = np.searchsorted(blk, np.arange(N_CORES + 1))
    counts = np.diff(bounds)
    # +1: last tile holds exactly the 128 self-loop edges (reuses block rows)
    T_main = max(1, int(math.ceil(counts.max() / P))) + 1
    T_main += T_main % 2  # round to even for compile-cache stability

    sel0 = dst_all == 0
    b0_src = src_all[sel0]
    b0_dst = dst_all[sel0]
    T0 = max(1, int(math.ceil(len(b0_src) / P)))
    bidx = _pack_edges(b0_src, b0_dst, T0)
    brows = np.arange(P, dtype=np.int32)

    nc = _get_program(n_nodes, in_dim, out_dim, T_main, T0)

    av = np.ascontiguousarray(np.stack([att_src, att_dst], axis=1))  # [OUTD, 2]
    in_maps = []
    for k in range(N_CORES):
        lo, hi = bounds[k], bounds[k + 1]
        mrows_flat = np.minimum(
            np.arange(k * P, (k + 1) * P, dtype=np.int32), n_nodes - 1
        )
        es = src_f[lo:hi]
        ed = dst_f[lo:hi] - k * P
        npad = (T_main - 1) * P - len(es)
        es = np.concatenate([es, np.zeros(npad, np.int32), mrows_flat])
        loop_dst = np.arange(P, dtype=np.int32)
        if (k + 1) * P > item_len:  # rows beyond item_len get no self-loop
            loop_dst = np.where(
                np.arange(k * P, (k + 1) * P) < item_len, loop_dst, P
            ).astype(np.int32)
        ed = np.concatenate([ed, np.full(npad, P, np.int32), loop_dst])
        eidx = _pack_edges(es, ed, T_main)
        in_maps.append(
            {
                "x_in": x,
                "w_in": W,
                "av_in": av,
                "bias_in": np.ascontiguousarray(bias.reshape(1, out_dim)),
                "eidx_in": eidx,
                "bidx_in": bidx,
                "rows_in": np.ascontiguousarray(
                    np.stack([mrows_flat, brows], axis=1)
                ),
            }
        )
    return nc, in_maps, item_len


def assemble(results, item_len):
    y_all = np.concatenate([results[k]["y_out"].ravel() for k in range(N_CORES)])
    return y_all[1:item_len].astype(np.float32)


def kernel(x, edge_index, W, att_src, att_dst, bias, item_len):
    from concourse import bass_utils

    nc, in_maps, item_len = prepare(
        x, edge_index, W, att_src, att_dst, bias, item_len
    )
    res = bass_utils.run_bass_kernel_spmd(nc, in_maps, core_ids=list(range(N_CORES)))
    return assemble(res.results, item_len)
